# revision 6
# baseline (speedup 1.0000x reference)
"""Distributed causal GQA attention kernel for one TRN2 chip (8 NeuronCores).

Problem: b=1, T=2048, d_model=2048, 32 Q heads, 8 KV heads, head_dim=64,
llama3-scaled RoPE, causal softmax attention, out-projection.

Sharding (tensor-parallel over heads):
  core i holds Q heads 4i..4i+3 and KV head i (GQA groups align exactly),
  plus Wo rows 256i..256(i+1) (i.e. output-column shard).

Schedule per core (single fused phase, PE kept continuously busy so the
hardware P-state ramps to 2.4GHz and stays there):
  - QKV projection rounds R0..R3 (one 512-token chunk each) are interleaved
    with attention pairs A0..A3: proj matmuls act as fillers inside the
    attention S->exp->PV pipeline (4 per k-block) so the PE never waits on
    ScalarE's exp even with a single score buffer.
  - A4..A7 run with a double-buffered score pool (2-block slack); the
    out-projections of chunk 0 are woven into A6/A7, the rest drain in the
    tail while the remaining AllGathers finish.
  - softmax denominator via ones-augmented V matmul; normalization with
    DVE reciprocal (ScalarE runs exp only; DMA issue moved off ScalarE
    after the first ~20us).
  - context (bf16) AllGather per head-pair on the GpSimd queue.
"""

import sys

for _p in ("/opt/trn_rl_repo", "/root/.axon_site/_ro/trn_rl_repo"):
    if _p not in sys.path:
        sys.path.append(_p)

import numpy as np

import concourse.bass as bass
import concourse.bacc as bacc
import concourse.tile as tile
import concourse.mybir as mybir
from concourse.bass_utils import run_bass_kernel_spmd

F32 = mybir.dt.float32
F32R = mybir.dt.float32r
BF16 = mybir.dt.bfloat16
EXP = mybir.ActivationFunctionType.Exp

N_CORES = 8
T = 2048          # sequence length
D = 2048          # model dim
HD = 64           # head dim
HPC = 4           # q heads per core
DLOC = HPC * HD   # 256 local q-head dims / Wo rows per core
QCW = 512         # q chunk width
NQC = T // QCW    # 4
KBW = 128         # k block width
NKB = T // KBW    # 16
NM = D // 128     # 16 contraction chunks
ROPE_BASE = 500000.0
FREQ_CONFIG = {"factor": 32.0, "low_freq_factor": 1.0, "high_freq_factor": 4.0,
               "original_context_length": 8192}


def _rope_tables(start_pos: int):
    fc = FREQ_CONFIG
    inv_freq = 1.0 / ROPE_BASE ** (np.arange(0, HD, 2, dtype=np.float32) / HD)
    low_wl = fc["original_context_length"] / fc["low_freq_factor"]
    high_wl = fc["original_context_length"] / fc["high_freq_factor"]
    wavelen = 2.0 * np.pi / inv_freq
    inv_l = np.where(wavelen > low_wl, inv_freq / fc["factor"], inv_freq)
    smooth = (fc["original_context_length"] / wavelen - fc["low_freq_factor"]) / (
        fc["high_freq_factor"] - fc["low_freq_factor"])
    smoothed = (1.0 - smooth) * (inv_freq / fc["factor"]) + smooth * inv_freq
    med = (wavelen <= low_wl) & (wavelen >= high_wl)
    inv_freq = np.where(med, smoothed, inv_l)
    pos = np.arange(start_pos, start_pos + T, dtype=np.float32)
    ang = pos[:, None] * inv_freq[None, :]
    ang = np.concatenate([ang, ang], axis=1)          # [T, 64]
    cos = np.cos(ang).astype(np.float32)
    sin = np.sin(ang).astype(np.float32)
    cosT = cos.T                                       # [64, T]
    # fold the rotate-half sign into sin: rows 0..31 multiply -x2, rows 32..63 +x1
    sinT_signed = np.concatenate([-sin[:, :32].T, sin[:, 32:].T], axis=0)
    return (np.ascontiguousarray(cosT),
            np.ascontiguousarray(sinT_signed))         # [64, T] each


def build_nc():
    nc = bacc.Bacc("TRN2", target_bir_lowering=False, debug=False,
                   num_devices=N_CORES)

    xT = nc.dram_tensor("xT", [D, T], BF16, kind="ExternalInput")
    wqT = nc.dram_tensor("wqT", [D, DLOC], BF16, kind="ExternalInput")
    wkvT = nc.dram_tensor("wkvT", [D, 128], BF16, kind="ExternalInput")
    woT = nc.dram_tensor("woT", [D, DLOC], BF16, kind="ExternalInput")
    cosT = nc.dram_tensor("cosT", [64, T], F32, kind="ExternalInput")
    sinT = nc.dram_tensor("sinT", [64, T], F32, kind="ExternalInput")
    mask2 = nc.dram_tensor("mask2", [128, 2 * KBW], F32, kind="ExternalInput")
    eye64 = nc.dram_tensor("eye64", [128, 64], F32, kind="ExternalInput")
    out = nc.dram_tensor("out", [DLOC, T], F32, kind="ExternalOutput")

    rg = [list(range(N_CORES))]

    with tile.TileContext(nc) as tc:
        with tc.tile_pool(name="wpool", bufs=1) as wpool, \
             tc.tile_pool(name="xpool", bufs=8) as xpool, \
             tc.tile_pool(name="rpool", bufs=6) as rpool, \
             tc.tile_pool(name="qpool", bufs=1) as qpool, \
             tc.tile_pool(name="ppool", bufs=4) as ppool, \
             tc.tile_pool(name="npool", bufs=6) as npool, \
             tc.tile_pool(name="agpool", bufs=3) as agpool, \
             tc.tile_pool(name="dram", bufs=1, space="DRAM") as dram:

            # ---- resident constants / weights ----
            cos_sb = wpool.tile([128, T], F32)
            sin_sb = wpool.tile([128, T], F32)
            mask_sb = wpool.tile([128, 2 * KBW], F32)
            eye_sb = wpool.tile([128, 64], F32)
            wq_sb = wpool.tile([128, NM, DLOC], BF16)
            wkv_sb = wpool.tile([128, NM, 128], BF16)
            wo_sb = wpool.tile([128, NM, DLOC], BF16)

            # warm collective fired immediately from a memset tile: the first
            # collective pays the one-time CC channel init (~65us); absorb it
            # during the projection phase.
            warm_src = wpool.tile([1, 16], F32)
            nc.gpsimd.memset(warm_src[0:1, :], 0.0)
            ag_warm_in = dram.tile([1, 16], F32, name="ag_warm_in")
            ag_warm_out = dram.tile([8, 16], F32, addr_space="Shared",
                                    name="ag_warm_out")
            nc.gpsimd.dma_start(out=ag_warm_in[:], in_=warm_src[0:1, :])
            nc.gpsimd.collective_compute(
                "AllGather", mybir.AluOpType.bypass, replica_groups=rg,
                ins=[ag_warm_in.opt()], outs=[ag_warm_out.opt()])

            # warm the Exp ACT table before the first real exp
            warm_sb = wpool.tile([1, 16], F32)
            nc.scalar.activation(warm_sb[0:1, :], warm_src[0:1, :], EXP,
                                 scale=0.125)

            # early small constants on GpSimd queue
            nc.gpsimd.dma_start(out=eye_sb[:], in_=eye64[:])
            nc.gpsimd.dma_start(out=mask_sb[:], in_=mask2[:])

            wq_r = wqT.rearrange("(m p) d -> p m d", p=128)
            wkv_r = wkvT.rearrange("(m p) d -> p m d", p=128)
            wo_r = woT.rearrange("(m p) d -> p m d", p=128)

            # ---- resident activations ----
            q_sb0 = qpool.tile([128, T], F32)    # pre-rope Q^T heads 0,1
            q_sb1 = qpool.tile([128, T], F32)    # pre-rope Q^T heads 2,3
            kv_sb = qpool.tile([128, T], F32)    # rows 0:64 K^T, 64:128 V^T
            qr0 = qpool.tile([128, T], F32R)     # rope'd Q^T heads 0,1
            qr1 = qpool.tile([128, T], F32R)     # rope'd Q^T heads 2,3
            kdup = qpool.tile([128, T], F32R)    # rope'd K^T on both halves
            vprime = [qpool.tile([128, 72], BF16, name=f"vp{b}")
                      for b in range(NKB)]

            # allgather buffers
            ag_in = {}
            ag_out = {}
            for c in range(NQC):
                for p in range(2):
                    ag_in[c, p] = dram.tile([128, QCW], BF16,
                                            name=f"ag_in_{c}_{p}")
                    ag_out[c, p] = dram.tile([1024, QCW], BF16,
                                             addr_space="Shared",
                                             name=f"ag_out_{c}_{p}")

            x_r = xT.rearrange("(m p) t -> p m t", p=128)

            with tc.tile_pool(name="cpsum", bufs=2, space="PSUM") as cpsum:

                # ---------- emission helpers ----------
                fillers = []          # list of zero-arg closures, 1 PE op each

                def pull_fillers(k):
                    n = 0
                    while fillers and n < k:
                        fillers.pop(0)()
                        n += 1

                proj_ps = {}          # n -> (kvps, qps0, qps1)
                xg_eng = [nc.sync, nc.scalar]

                def alloc_xgs(n):
                    return [xpool.tile([128, 4, QCW], BF16, tag="xg",
                                       name=f"xg_{n}_{g}") for g in range(4)]

                def emit_xg_dma(n, xgs, g):
                    ql = QCW * n
                    xg_eng[g % 2].dma_start(
                        out=xgs[g][:],
                        in_=x_r[:, 4 * g:4 * g + 4, ql:ql + QCW])

                def make_proj_closures(projps, n, xgs):
                    """48 matmul closures for proj round n."""
                    kvps = projps.tile([128, QCW], F32, tag="proj",
                                       name=f"kvps_{n}")
                    qps0 = projps.tile([128, QCW], F32, tag="proj",
                                       name=f"qps0_{n}")
                    qps1 = projps.tile([128, QCW], F32, tag="proj",
                                       name=f"qps1_{n}")
                    proj_ps[n] = (kvps, qps0, qps1)
                    cls = []
                    for m in range(NM):
                        first, last = (m == 0), (m == NM - 1)

                        def f(m=m, first=first, last=last):
                            xg = xgs[m // 4][:, m % 4, :]
                            nc.tensor.matmul(kvps[:], wkv_sb[:, m, :],
                                             xg, start=first, stop=last)
                            nc.tensor.matmul(qps0[:], wq_sb[:, m, 0:128],
                                             xg, start=first, stop=last)
                            nc.tensor.matmul(qps1[:], wq_sb[:, m, 128:256],
                                             xg, start=first, stop=last)
                        cls.append(f)
                    return cls

                def emit_copies_rope(n):
                    """PSUM->SBUF copies + RoPE for round n (Vector/GpSimd)."""
                    ql = QCW * n
                    kvps, qps0, qps1 = proj_ps.pop(n)
                    # K first: attention S of this chunk's blocks needs kdup
                    nc.vector.tensor_copy(kv_sb[:, ql:ql + QCW], kvps[:])
                    rotk = rpool.tile([128, QCW], F32, tag="rope",
                                      name=f"rope_{n}_k")
                    nc.gpsimd.dma_start(out=rotk[0:32, :],
                                        in_=kv_sb[32:64, ql:ql + QCW])
                    nc.gpsimd.dma_start(out=rotk[32:64, :],
                                        in_=kv_sb[0:32, ql:ql + QCW])
                    nc.vector.tensor_mul(rotk[0:64, :], rotk[0:64, :],
                                         sin_sb[0:64, ql:ql + QCW])
                    nc.vector.tensor_mul(kv_sb[0:64, ql:ql + QCW],
                                         kv_sb[0:64, ql:ql + QCW],
                                         cos_sb[0:64, ql:ql + QCW])
                    nc.vector.tensor_add(kdup[0:64, ql:ql + QCW],
                                         kv_sb[0:64, ql:ql + QCW],
                                         rotk[0:64, :])
                    nc.gpsimd.dma_start(out=kdup[64:128, ql:ql + QCW],
                                        in_=kdup[0:64, ql:ql + QCW])
                    # Q pairs
                    for qsb, qps, qr, idx in ((q_sb0, qps0, qr0, 0),
                                              (q_sb1, qps1, qr1, 1)):
                        nc.vector.tensor_copy(qsb[:, ql:ql + QCW], qps[:])
                        rot = rpool.tile([128, QCW], F32, tag="rope",
                                         name=f"rope_{n}_{idx}")
                        for h in range(2):
                            nc.gpsimd.dma_start(
                                out=rot[64 * h:64 * h + 32, :],
                                in_=qsb[64 * h + 32:64 * h + 64, ql:ql + QCW])
                            nc.gpsimd.dma_start(
                                out=rot[64 * h + 32:64 * h + 64, :],
                                in_=qsb[64 * h:64 * h + 32, ql:ql + QCW])
                        nc.vector.tensor_mul(rot[:], rot[:],
                                             sin_sb[:, ql:ql + QCW])
                        nc.vector.tensor_mul(qsb[:, ql:ql + QCW],
                                             qsb[:, ql:ql + QCW],
                                             cos_sb[:, ql:ql + QCW])
                        nc.vector.tensor_add(qr[:, ql:ql + QCW],
                                             qsb[:, ql:ql + QCW], rot[:])

                def make_vt_closures(tps, blocks):
                    """V transpose closures (1 PE transpose each)."""
                    cls = []
                    for b in blocks:
                        def f(b=b):
                            vt_ps = tps.tile([128, 64], F32, tag="vt",
                                             name=f"vtps_{b}")
                            nc.tensor.transpose(
                                vt_ps[:],
                                kv_sb[64:128, KBW * b:KBW * (b + 1)],
                                eye_sb[64:128, :])
                            nc.vector.tensor_copy(vprime[b][:, 0:64],
                                                  vt_ps[:])
                            nc.vector.memset(
                                vprime[b][:, 64:66].bitcast(mybir.dt.uint16),
                                0x3F80)
                        cls.append(f)
                    return cls

                def qr_rhs(h, lo, hi):
                    t_ = qr0 if h < 2 else qr1
                    base = 64 * (h % 2)
                    return t_[base:base + 64, lo:hi]

                def k_lhs(h, b):
                    base = 64 * (h % 2)
                    return kdup[base:base + 64, KBW * b:KBW * (b + 1)]

                def emit_attention_pair(spsum, c, p, fpb):
                    """Attention pair (c,p): per k-block S (2 heads into one
                    [128,1024] pair tile) -> exp -> PV, pulling fpb(b)
                    fillers per block."""
                    if not callable(fpb):
                        kk = fpb

                        def fpb(b, kk=kk):
                            return kk
                    ql = QCW * c
                    nb = 4 * c + 4
                    ctx2 = [cpsum.tile([128, QCW], F32, tag="cps",
                                       name=f"ctxp_{c}_{p}_{hh}")
                            for hh in range(2)]
                    pend = []   # (b, pp, qs)

                    def emit_pv():
                        b, pp, qs = pend.pop(0)
                        first = (b == 0)
                        last = (b == nb - 1)
                        for hh in range(2):
                            nc.tensor.matmul(
                                ctx2[hh][0:66, qs:QCW], vprime[b][:, 0:66],
                                pp[:, QCW * hh + qs:QCW * (hh + 1)],
                                start=first, stop=last)

                    for b in range(nb):
                        diag = b >= 4 * c
                        qs = KBW * (b - 4 * c) if diag else 0
                        sp = spsum.tile([128, 2 * QCW], F32, tag="spair",
                                        name=f"sp_{c}_{p}_{b}")
                        for hh in range(2):
                            h = 2 * p + hh
                            nc.tensor.matmul(
                                sp[:, QCW * hh + qs:QCW * (hh + 1)],
                                k_lhs(h, b), qr_rhs(h, ql + qs, ql + QCW),
                                start=True, stop=True)
                        if diag:
                            sv = sp[:].rearrange("p (h q) -> p h q", h=2)[
                                :, :, qs:qs + KBW]
                            mv = mask_sb[:, 0:2 * KBW].rearrange(
                                "p (h q) -> p h q", h=2)
                            nc.vector.tensor_add(sv, sv, mv)
                        pp = ppool.tile([128, 2 * QCW], BF16, tag="pp",
                                        name=f"pp_{c}_{p}_{b}")
                        if qs == 0:
                            nc.scalar.activation(pp[:], sp[:], EXP,
                                                 scale=0.125)
                        else:
                            for hh in range(2):
                                nc.scalar.activation(
                                    pp[:, QCW * hh + qs:QCW * (hh + 1)],
                                    sp[:, QCW * hh + qs:QCW * (hh + 1)],
                                    EXP, scale=0.125)
                        pend.append((b, pp, qs))
                        pull_fillers(fpb(b))
                        if len(pend) >= 2:
                            emit_pv()
                    while pend:
                        pull_fillers(1)
                        emit_pv()

                    # normalize + ship + allgather
                    for hh in range(2):
                        h = 2 * p + hh
                        rr = npool.tile([1, QCW], F32, tag="rr",
                                        name=f"rr_{c}_{h}")
                        nc.vector.reciprocal(rr[0:1, :], ctx2[hh][64:65, :])
                        rb = npool.tile([64, QCW], F32, tag="rb",
                                        name=f"rb_{c}_{h}")
                        nc.gpsimd.partition_broadcast(rb[0:64, :], rr[0:1, :])
                        cn = npool.tile([64, QCW], BF16, tag="cn",
                                        name=f"cn_{c}_{h}")
                        nc.vector.tensor_mul(cn[0:64, :], ctx2[hh][0:64, :],
                                             rb[0:64, :])
                        nc.gpsimd.dma_start(
                            out=ag_in[c, p][64 * hh:64 * (hh + 1), :],
                            in_=cn[0:64, :])
                    nc.gpsimd.collective_compute(
                        "AllGather", mybir.AluOpType.bypass,
                        replica_groups=rg,
                        ins=[ag_in[c, p].opt()], outs=[ag_out[c, p].opt()])

                # ---------- out-projection ----------
                outproj_state = {}

                def make_outproj_closures(opsum, c, p):
                    """16 matmul closures (2 gg-groups) for pair (c,p).
                    agt loads are issued NOW (Sync prefetch behind the AG)."""
                    if c not in outproj_state:
                        outproj_state[c] = [
                            opsum.tile([128, QCW], F32, tag="ops",
                                       name=f"ops_{c}_{cb}")
                            for cb in range(2)]
                    ops = outproj_state[c]
                    agts = []
                    for gg in range(2):
                        agt = agpool.tile([128, 4, QCW], BF16, tag="ag",
                                          name=f"agt_{c}_{p}_{gg}")
                        nc.sync.dma_start(
                            out=agt[:],
                            in_=ag_out[c, p].rearrange(
                                "(g q) t -> q g t",
                                q=128)[:, 4 * gg:4 * gg + 4, :])
                        agts.append(agt)
                    cls = []
                    for gg in range(2):
                        for gl in range(4):
                            g = 4 * gg + gl
                            m = 2 * g + p
                            for cb in range(2):
                                def f(gg=gg, gl=gl, m=m, cb=cb, g=g):
                                    nc.tensor.matmul(
                                        ops[cb][:],
                                        wo_sb[:, m, 128 * cb:128 * (cb + 1)],
                                        agts[gg][:, gl, :],
                                        start=(p == 0 and g == 0),
                                        stop=(p == 1 and g == 7))
                                cls.append(f)
                    return cls

                def emit_outproj_finish(c):
                    ops = outproj_state.pop(c)
                    for cb in range(2):
                        osb = npool.tile([128, QCW], F32, tag="osb",
                                         name=f"osb_{c}_{cb}")
                        nc.vector.tensor_copy(osb[:], ops[cb][:])
                        nc.gpsimd.dma_start(
                            out=out[128 * cb:128 * (cb + 1),
                                    QCW * c:QCW * (c + 1)],
                            in_=osb[:])

                # ---------- the schedule ----------
                # scope 1: R0..A3 -- proj 3 banks + vt + spA 2 + cps 2
                with tc.tile_pool(name="projps", bufs=3, space="PSUM") \
                        as projps, \
                     tc.tile_pool(name="tps", bufs=1, space="PSUM") as tps, \
                     tc.tile_pool(name="spsumA", bufs=1, space="PSUM") \
                        as spsumA:

                    # round 0: weights + xg interleaved on Sync/Scalar
                    xgs0 = alloc_xgs(0)
                    r0 = make_proj_closures(projps, 0, xgs0)
                    nc.scalar.dma_start(out=wq_sb[:, 0:4, :],
                                        in_=wq_r[:, 0:4, :])
                    nc.sync.dma_start(out=wkv_sb[:, 0:4, :],
                                      in_=wkv_r[:, 0:4, :])
                    emit_xg_dma(0, xgs0, 0)
                    emit_xg_dma(0, xgs0, 1)     # scalar
                    nc.sync.dma_start(out=wkv_sb[:, 4:8, :],
                                      in_=wkv_r[:, 4:8, :])
                    nc.scalar.dma_start(out=wq_sb[:, 4:8, :],
                                        in_=wq_r[:, 4:8, :])
                    emit_xg_dma(0, xgs0, 2)
                    nc.sync.dma_start(out=wkv_sb[:, 8:12, :],
                                      in_=wkv_r[:, 8:12, :])
                    emit_xg_dma(0, xgs0, 3)     # scalar
                    nc.scalar.dma_start(out=cos_sb[0:64, :], in_=cosT[:])
                    nc.sync.dma_start(out=wkv_sb[:, 12:16, :],
                                      in_=wkv_r[:, 12:16, :])
                    nc.scalar.dma_start(out=sin_sb[0:64, :], in_=sinT[:])
                    nc.sync.dma_start(out=wq_sb[:, 8:12, :],
                                      in_=wq_r[:, 8:12, :])
                    nc.scalar.dma_start(out=wq_sb[:, 12:16, :],
                                        in_=wq_r[:, 12:16, :])
                    nc.gpsimd.dma_start(out=cos_sb[64:128, :],
                                        in_=cos_sb[0:64, :])
                    nc.gpsimd.dma_start(out=sin_sb[64:128, :],
                                        in_=sin_sb[0:64, :])
                    for f in r0:
                        f()
                    emit_copies_rope(0)

                    xgs1 = alloc_xgs(1)
                    for g in range(4):
                        emit_xg_dma(1, xgs1, g)
                    r1 = make_proj_closures(projps, 1, xgs1)
                    for f in r1[:8]:
                        f()
                    fillers.extend(make_vt_closures(tps, range(0, 4)))
                    fillers.extend(r1[8:])
                    pull_fillers(4)

                    emit_attention_pair(spsumA, 0, 0, fpb=4)
                    pull_fillers(99)          # drain R1
                    emit_copies_rope(1)

                    xgs2 = alloc_xgs(2)
                    for g in range(4):
                        emit_xg_dma(2, xgs2, g)
                    fillers.extend(make_proj_closures(projps, 2, xgs2))
                    # wo chunks now (needed from A6 on)
                    for g in range(4):
                        sl = slice(4 * g, 4 * g + 4)
                        nc.sync.dma_start(out=wo_sb[:, sl, :],
                                          in_=wo_r[:, sl, :])
                    emit_attention_pair(spsumA, 0, 1, fpb=4)
                    # keep r2 remainder (32) as A2 fillers; vt4-7 first
                    fillers[:0] = make_vt_closures(tps, range(4, 8))
                    emit_attention_pair(spsumA, 1, 0, fpb=4)
                    pull_fillers(99)          # drain R2 leftovers
                    emit_copies_rope(2)
                    fillers.extend(make_vt_closures(tps, range(8, 12)))

                    xgs3 = alloc_xgs(3)
                    for g in range(4):
                        emit_xg_dma(3, xgs3, g)
                    fillers.extend(make_proj_closures(projps, 3, xgs3))
                    emit_attention_pair(spsumA, 1, 1, fpb=4)
                    pull_fillers(99)          # drain R3
                    emit_copies_rope(3)
                    for f in make_vt_closures(tps, range(12, 16)):
                        f()

                # scope 2: A4..tail -- spB 4 banks + ops 2 + cps 2
                with tc.tile_pool(name="spsumB", bufs=2, space="PSUM") \
                        as spsumB, \
                     tc.tile_pool(name="opsum", bufs=2, space="PSUM") \
                        as opsum:
                    emit_attention_pair(spsumB, 2, 0, fpb=0)
                    emit_attention_pair(spsumB, 2, 1, fpb=0)
                    fillers.extend(make_outproj_closures(opsum, 0, 0))
                    emit_attention_pair(spsumB, 3, 0,
                                        fpb=lambda b: 0 if b < 8 else 2)
                    pull_fillers(99)
                    fillers.extend(make_outproj_closures(opsum, 0, 1))
                    emit_attention_pair(spsumB, 3, 1,
                                        fpb=lambda b: 0 if b < 4 else 2)
                    pull_fillers(99)
                    emit_outproj_finish(0)
                    for f in make_outproj_closures(opsum, 1, 0):
                        f()
                    for f in make_outproj_closures(opsum, 1, 1):
                        f()
                    emit_outproj_finish(1)
                    for f in make_outproj_closures(opsum, 2, 0):
                        f()
                    for f in make_outproj_closures(opsum, 2, 1):
                        f()
                    emit_outproj_finish(2)
                    for f in make_outproj_closures(opsum, 3, 0):
                        f()
                    for f in make_outproj_closures(opsum, 3, 1):
                        f()
                    emit_outproj_finish(3)

    nc.compile()
    return nc


_NC_CACHE = None


def _get_nc():
    global _NC_CACHE
    if _NC_CACHE is None:
        _NC_CACHE = build_nc()
    return _NC_CACHE


def _build_in_maps(inputs):
    import ml_dtypes
    x = np.asarray(inputs["x"], dtype=np.float32)
    Wq = np.asarray(inputs["Wq"], dtype=np.float32)
    Wk = np.asarray(inputs["Wk"], dtype=np.float32)
    Wv = np.asarray(inputs["Wv"], dtype=np.float32)
    Wo = np.asarray(inputs["Wo"], dtype=np.float32)
    sp = int(np.asarray(inputs["start_pos"]))

    b, t, d = x.shape
    assert (b, t, d) == (1, T, D), (b, t, d)

    cosT_np, sinT_np = _rope_tables(sp)
    xT_r = np.ascontiguousarray(x[0].T).astype(ml_dtypes.bfloat16)  # [D, T]
    tri = np.where(np.arange(KBW)[:, None] > np.arange(KBW)[None, :],
                   np.float32(-1e30), np.float32(0.0))
    mask2_np = np.ascontiguousarray(np.tile(tri, (1, 2)))  # [128, 256]
    eye_np = np.tile(np.eye(64, dtype=np.float32), (2, 1))  # [128, 64]

    in_maps = []
    for i in range(N_CORES):
        wqT_i = np.ascontiguousarray(
            Wq[DLOC * i:DLOC * (i + 1), :].T).astype(ml_dtypes.bfloat16)
        wkv_i = np.ascontiguousarray(
            np.concatenate([Wk[HD * i:HD * (i + 1), :].T,
                            Wv[HD * i:HD * (i + 1), :].T],
                           axis=1)).astype(ml_dtypes.bfloat16)  # [D, 128]
        woT_i = np.ascontiguousarray(
            Wo[DLOC * i:DLOC * (i + 1), :].T).astype(ml_dtypes.bfloat16)
        in_maps.append({
            "xT": xT_r,
            "wqT": wqT_i,
            "wkvT": wkv_i,
            "woT": woT_i,
            "cosT": cosT_np,
            "sinT": sinT_np,
            "mask2": mask2_np,
            "eye64": eye_np,
        })
    return in_maps


def kernel(x, Wq, Wk, Wv, Wo, start_pos):
    in_maps = _build_in_maps(dict(x=x, Wq=Wq, Wk=Wk, Wv=Wv, Wo=Wo,
                                  start_pos=start_pos))
    nc = _get_nc()
    res = run_bass_kernel_spmd(nc, in_maps, core_ids=list(range(N_CORES)))

    outT = np.empty((T, D), dtype=np.float32)
    for i in range(N_CORES):
        outT[:, DLOC * i:DLOC * (i + 1)] = res.results[i]["out"].T
    return outT[None, :, :]


if __name__ == "__main__":
    rng = np.random.default_rng(0)
    inputs = {
        "x": rng.standard_normal((1, T, D)).astype(np.float32),
        "Wq": (rng.standard_normal((D, D)) * 0.02).astype(np.float32),
        "Wk": (rng.standard_normal((512, D)) * 0.02).astype(np.float32),
        "Wv": (rng.standard_normal((512, D)) * 0.02).astype(np.float32),
        "Wo": (rng.standard_normal((D, D)) * 0.02).astype(np.float32),
        "start_pos": 0,
    }
    y = kernel(**inputs)
    print("kernel output shape:", y.shape, "finite:", np.isfinite(y).all())


# revision 11
# speedup vs baseline: 1.1194x; 1.1194x over previous
"""Distributed causal GQA attention kernel for one TRN2 chip (8 NeuronCores).

Problem: b=1, T=2048, d_model=2048, 32 Q heads, 8 KV heads, head_dim=64,
llama3-scaled RoPE, causal softmax attention, out-projection.

Sharding (tensor-parallel over heads):
  core i holds Q heads 4i..4i+3 and KV head i (GQA groups align exactly),
  plus Wo rows 256i..256(i+1) (i.e. output-column shard).

Schedule per core (single fused phase, PE kept continuously busy so the
hardware P-state ramps to 2.4GHz and stays there):
  - QKV projection rounds R0..R3 (one 512-token chunk each) are interleaved
    with attention pairs A0..A3: proj matmuls act as fillers inside the
    attention S->exp->PV pipeline (4 per k-block) so the PE never waits on
    ScalarE's exp even with a single score buffer.
  - A4..A7 run with a double-buffered score pool (2-block slack); the
    out-projections of chunk 0 are woven into A6/A7, the rest drain in the
    tail while the remaining AllGathers finish.
  - softmax denominator via ones-augmented V matmul; normalization with
    DVE reciprocal (ScalarE runs exp only; DMA issue moved off ScalarE
    after the first ~20us).
  - context (bf16) AllGather per head-pair on the GpSimd queue.
"""

import sys

for _p in ("/opt/trn_rl_repo", "/root/.axon_site/_ro/trn_rl_repo"):
    if _p not in sys.path:
        sys.path.append(_p)

import numpy as np

import concourse.bass as bass
import concourse.bacc as bacc
import concourse.tile as tile
import concourse.mybir as mybir
from concourse.bass_utils import run_bass_kernel_spmd

F32 = mybir.dt.float32
F32R = mybir.dt.float32r
BF16 = mybir.dt.bfloat16
EXP = mybir.ActivationFunctionType.Exp
LOG = mybir.ActivationFunctionType.Ln


def _patch_activation_tables():
    """Make Exp and Ln resolve only to the combined natural_log_exp set so
    the table-load pass emits one load instead of thrashing between the
    exp-only and ln-only sets (2.7us per switch, mid-attention)."""
    import functools
    from concourse.hw_specs import get_activation_tables as orig

    @functools.cache
    def patched(arch):
        tables = dict(orig(arch))
        comb = "natural_log_exp_and_others"
        if comb not in tables:
            return tables
        exp_ln = {mybir.ActivationFunctionType.Exp,
                  mybir.ActivationFunctionType.Ln}
        return {name: (funcs if name == comb else funcs - exp_ln)
                for name, funcs in tables.items()}

    bacc.get_activation_tables = patched

N_CORES = 8
T = 2048          # sequence length
D = 2048          # model dim
HD = 64           # head dim
HPC = 4           # q heads per core
DLOC = HPC * HD   # 256 local q-head dims / Wo rows per core
QCW = 512         # q chunk width
NQC = T // QCW    # 4
KBW = 128         # k block width
NKB = T // KBW    # 16
NM = D // 128     # 16 contraction chunks
ROPE_BASE = 500000.0
FREQ_CONFIG = {"factor": 32.0, "low_freq_factor": 1.0, "high_freq_factor": 4.0,
               "original_context_length": 8192}


def _rope_tables(start_pos: int):
    fc = FREQ_CONFIG
    inv_freq = 1.0 / ROPE_BASE ** (np.arange(0, HD, 2, dtype=np.float32) / HD)
    low_wl = fc["original_context_length"] / fc["low_freq_factor"]
    high_wl = fc["original_context_length"] / fc["high_freq_factor"]
    wavelen = 2.0 * np.pi / inv_freq
    inv_l = np.where(wavelen > low_wl, inv_freq / fc["factor"], inv_freq)
    smooth = (fc["original_context_length"] / wavelen - fc["low_freq_factor"]) / (
        fc["high_freq_factor"] - fc["low_freq_factor"])
    smoothed = (1.0 - smooth) * (inv_freq / fc["factor"]) + smooth * inv_freq
    med = (wavelen <= low_wl) & (wavelen >= high_wl)
    inv_freq = np.where(med, smoothed, inv_l)
    pos = np.arange(start_pos, start_pos + T, dtype=np.float32)
    ang = pos[:, None] * inv_freq[None, :]
    ang = np.concatenate([ang, ang], axis=1)          # [T, 64]
    cos = np.cos(ang).astype(np.float32)
    sin = np.sin(ang).astype(np.float32)
    cosT = cos.T                                       # [64, T]
    # fold the rotate-half sign into sin: rows 0..31 multiply -x2, rows 32..63 +x1
    sinT_signed = np.concatenate([-sin[:, :32].T, sin[:, 32:].T], axis=0)
    return (np.ascontiguousarray(cosT),
            np.ascontiguousarray(sinT_signed))         # [64, T] each


def build_nc():
    _patch_activation_tables()
    nc = bacc.Bacc("TRN2", target_bir_lowering=False, debug=False,
                   num_devices=N_CORES)

    xT = nc.dram_tensor("xT", [D, T], BF16, kind="ExternalInput")
    wqT = nc.dram_tensor("wqT", [D, DLOC], BF16, kind="ExternalInput")
    wkvT = nc.dram_tensor("wkvT", [D, 128], BF16, kind="ExternalInput")
    woT = nc.dram_tensor("woT", [D, DLOC], BF16, kind="ExternalInput")
    cosT = nc.dram_tensor("cosT", [64, T], F32, kind="ExternalInput")
    sinT = nc.dram_tensor("sinT", [64, T], F32, kind="ExternalInput")
    mask2 = nc.dram_tensor("mask2", [128, 2 * KBW], F32, kind="ExternalInput")
    eye64 = nc.dram_tensor("eye64", [128, 64], F32, kind="ExternalInput")
    out = nc.dram_tensor("out", [DLOC, T], F32, kind="ExternalOutput")

    rg = [list(range(N_CORES))]

    with tile.TileContext(nc) as tc:
        with tc.tile_pool(name="wpool", bufs=1) as wpool, \
             tc.tile_pool(name="xpool", bufs=12) as xpool, \
             tc.tile_pool(name="rpool", bufs=6) as rpool, \
             tc.tile_pool(name="qpool", bufs=1) as qpool, \
             tc.tile_pool(name="ppool", bufs=4) as ppool, \
             tc.tile_pool(name="npool", bufs=4) as npool, \
             tc.tile_pool(name="agpool", bufs=3) as agpool, \
             tc.tile_pool(name="dram", bufs=1, space="DRAM") as dram:

            # ---- resident constants / weights ----
            cos_sb = wpool.tile([128, T], F32)
            sin_sb = wpool.tile([128, T], F32)
            mask_sb = wpool.tile([128, 2 * KBW], F32)
            eye_sb = wpool.tile([128, 64], F32)
            wq_sb = wpool.tile([128, NM, DLOC], BF16)
            wkv_sb = wpool.tile([128, NM, 128], BF16)
            wo_sb = wpool.tile([128, NM, DLOC], BF16)

            # warm collective fired immediately from a memset tile: the first
            # collective pays the one-time CC channel init (~65us); absorb it
            # during the projection phase.
            warm_src = wpool.tile([1, 16], F32)
            nc.gpsimd.memset(warm_src[0:1, :], 0.0)
            ag_warm_in = dram.tile([1, 16], F32, name="ag_warm_in")
            ag_warm_out = dram.tile([8, 16], F32, addr_space="Shared",
                                    name="ag_warm_out")
            nc.gpsimd.dma_start(out=ag_warm_in[:], in_=warm_src[0:1, :])
            nc.gpsimd.collective_compute(
                "AllGather", mybir.AluOpType.bypass, replica_groups=rg,
                ins=[ag_warm_in.opt()], outs=[ag_warm_out.opt()])

            # warm the combined Ln+Exp ACT table before the first real exp
            warm_sb = wpool.tile([1, 16], F32)
            nc.scalar.activation(warm_sb[0:1, :], warm_src[0:1, :], LOG,
                                 bias=1.0, scale=1.0)
            nc.scalar.activation(warm_sb[0:1, :], warm_src[0:1, :], EXP,
                                 scale=0.125)

            # early small constants on GpSimd queue
            nc.gpsimd.dma_start(out=eye_sb[:], in_=eye64[:])
            nc.gpsimd.dma_start(out=mask_sb[:], in_=mask2[:])

            wq_r = wqT.rearrange("(m p) d -> p m d", p=128)
            wkv_r = wkvT.rearrange("(m p) d -> p m d", p=128)
            wo_r = woT.rearrange("(m p) d -> p m d", p=128)

            # ---- resident activations ----
            q_sb0 = qpool.tile([128, T], F32)    # pre-rope Q^T heads 0,1
            q_sb1 = qpool.tile([128, T], F32)    # pre-rope Q^T heads 2,3
            kv_sb = qpool.tile([128, T], F32)    # rows 0:64 K^T, 64:128 V^T
            qr0 = qpool.tile([128, T], F32R)     # rope'd Q^T heads 0,1
            qr1 = qpool.tile([128, T], F32R)     # rope'd Q^T heads 2,3
            kdup = qpool.tile([128, T], F32R)    # rope'd K^T on both halves
            vprime = [qpool.tile([128, 72], BF16, name=f"vp{b}")
                      for b in range(NKB)]

            # allgather buffers
            ag_in = {}
            ag_out = {}
            for c in range(NQC):
                for p in range(2):
                    ag_in[c, p] = dram.tile([128, QCW], BF16,
                                            name=f"ag_in_{c}_{p}")
                    ag_out[c, p] = dram.tile([1024, QCW], BF16,
                                             addr_space="Shared",
                                             name=f"ag_out_{c}_{p}")

            x_r = xT.rearrange("(m p) t -> p m t", p=128)

            with tc.tile_pool(name="cpsum", bufs=2, space="PSUM") as cpsum:

                # ---------- emission helpers ----------
                fillers = []          # list of zero-arg closures, 1 PE op each

                def pull_fillers(k):
                    n = 0
                    while fillers and n < k:
                        fillers.pop(0)()
                        n += 1

                proj_ps = {}          # n -> (kvps, qps0, qps1)
                xg_eng = [nc.sync, nc.scalar]

                def alloc_xgs(n):
                    return [xpool.tile([128, 4, QCW], BF16, tag="xg",
                                       name=f"xg_{n}_{g}") for g in range(4)]

                def emit_xg_dma(n, xgs, g):
                    ql = QCW * n
                    xg_eng[g % 2].dma_start(
                        out=xgs[g][:],
                        in_=x_r[:, 4 * g:4 * g + 4, ql:ql + QCW])

                def make_proj_closures(projps, n, xgs):
                    """48 matmul closures for proj round n."""
                    kvps = projps.tile([128, QCW], F32, tag="proj",
                                       name=f"kvps_{n}")
                    qps0 = projps.tile([128, QCW], F32, tag="proj",
                                       name=f"qps0_{n}")
                    qps1 = projps.tile([128, QCW], F32, tag="proj",
                                       name=f"qps1_{n}")
                    proj_ps[n] = (kvps, qps0, qps1)
                    cls = []
                    for m in range(NM):
                        first, last = (m == 0), (m == NM - 1)

                        def f(m=m, first=first, last=last):
                            xg = xgs[m // 4][:, m % 4, :]
                            nc.tensor.matmul(kvps[:], wkv_sb[:, m, :],
                                             xg, start=first, stop=last)
                            nc.tensor.matmul(qps0[:], wq_sb[:, m, 0:128],
                                             xg, start=first, stop=last)
                            nc.tensor.matmul(qps1[:], wq_sb[:, m, 128:256],
                                             xg, start=first, stop=last)
                        cls.append(f)
                    return cls

                def emit_copies_rope(n):
                    """PSUM->SBUF copies + RoPE for round n (Vector/GpSimd)."""
                    ql = QCW * n
                    kvps, qps0, qps1 = proj_ps.pop(n)
                    # K first: attention S of this chunk's blocks needs kdup
                    nc.vector.tensor_copy(kv_sb[:, ql:ql + QCW], kvps[:])
                    rotk = rpool.tile([128, QCW], F32, tag="rope",
                                      name=f"rope_{n}_k")
                    nc.gpsimd.dma_start(out=rotk[0:32, :],
                                        in_=kv_sb[32:64, ql:ql + QCW])
                    nc.gpsimd.dma_start(out=rotk[32:64, :],
                                        in_=kv_sb[0:32, ql:ql + QCW])
                    nc.vector.tensor_mul(rotk[0:64, :], rotk[0:64, :],
                                         sin_sb[0:64, ql:ql + QCW])
                    nc.vector.tensor_mul(kv_sb[0:64, ql:ql + QCW],
                                         kv_sb[0:64, ql:ql + QCW],
                                         cos_sb[0:64, ql:ql + QCW])
                    nc.vector.tensor_add(kdup[0:64, ql:ql + QCW],
                                         kv_sb[0:64, ql:ql + QCW],
                                         rotk[0:64, :])
                    nc.gpsimd.dma_start(out=kdup[64:128, ql:ql + QCW],
                                        in_=kdup[0:64, ql:ql + QCW])
                    # Q pairs
                    for qsb, qps, qr, idx in ((q_sb0, qps0, qr0, 0),
                                              (q_sb1, qps1, qr1, 1)):
                        nc.vector.tensor_copy(qsb[:, ql:ql + QCW], qps[:])
                        rot = rpool.tile([128, QCW], F32, tag="rope",
                                         name=f"rope_{n}_{idx}")
                        for h in range(2):
                            nc.gpsimd.dma_start(
                                out=rot[64 * h:64 * h + 32, :],
                                in_=qsb[64 * h + 32:64 * h + 64, ql:ql + QCW])
                            nc.gpsimd.dma_start(
                                out=rot[64 * h + 32:64 * h + 64, :],
                                in_=qsb[64 * h:64 * h + 32, ql:ql + QCW])
                        nc.vector.tensor_mul(rot[:], rot[:],
                                             sin_sb[:, ql:ql + QCW])
                        nc.vector.tensor_mul(qsb[:, ql:ql + QCW],
                                             qsb[:, ql:ql + QCW],
                                             cos_sb[:, ql:ql + QCW])
                        nc.vector.tensor_add(qr[:, ql:ql + QCW],
                                             qsb[:, ql:ql + QCW], rot[:])

                def make_vt_closures(tps, blocks):
                    """V transpose closures (1 PE transpose each)."""
                    cls = []
                    for b in blocks:
                        def f(b=b):
                            vt_ps = tps.tile([128, 64], F32, tag="vt",
                                             name=f"vtps_{b}")
                            nc.tensor.transpose(
                                vt_ps[:],
                                kv_sb[64:128, KBW * b:KBW * (b + 1)],
                                eye_sb[64:128, :])
                            nc.vector.tensor_copy(vprime[b][:, 0:64],
                                                  vt_ps[:])
                            nc.vector.memset(
                                vprime[b][:, 64:66].bitcast(mybir.dt.uint16),
                                0x3F80)
                        cls.append(f)
                    return cls

                def qr_rhs(h, lo, hi):
                    t_ = qr0 if h < 2 else qr1
                    base = 64 * (h % 2)
                    return t_[base:base + 64, lo:hi]

                def k_lhs(h, b):
                    base = 64 * (h % 2)
                    return kdup[base:base + 64, KBW * b:KBW * (b + 1)]

                def emit_attention_pair(spsum, c, p, fpb):
                    """Attention pair (c,p): per k-block S (2 heads into one
                    [128,1024] pair tile) -> exp -> PV, pulling fpb(b)
                    fillers per block."""
                    if not callable(fpb):
                        kk = fpb

                        def fpb(b, kk=kk):
                            return kk
                    ql = QCW * c
                    nb = 4 * c + 4
                    ctx2 = [cpsum.tile([128, QCW], F32, tag="cps",
                                       name=f"ctxp_{c}_{p}_{hh}")
                            for hh in range(2)]
                    pend = []   # (b, pp, qs)

                    def emit_pv():
                        b, pp, qs = pend.pop(0)
                        first = (b == 0)
                        last = (b == nb - 1)
                        for hh in range(2):
                            nc.tensor.matmul(
                                ctx2[hh][0:66, qs:QCW], vprime[b][:, 0:66],
                                pp[:, QCW * hh + qs:QCW * (hh + 1)],
                                start=first, stop=last)

                    for b in range(nb):
                        diag = b >= 4 * c
                        qs = KBW * (b - 4 * c) if diag else 0
                        sp = spsum.tile([128, 2 * QCW], F32, tag="spair",
                                        name=f"sp_{c}_{p}_{b}")
                        for hh in range(2):
                            h = 2 * p + hh
                            nc.tensor.matmul(
                                sp[:, QCW * hh + qs:QCW * (hh + 1)],
                                k_lhs(h, b), qr_rhs(h, ql + qs, ql + QCW),
                                start=True, stop=True)
                        if diag:
                            sv = sp[:].rearrange("p (h q) -> p h q", h=2)[
                                :, :, qs:qs + KBW]
                            mv = mask_sb[:, 0:2 * KBW].rearrange(
                                "p (h q) -> p h q", h=2)
                            nc.vector.tensor_add(sv, sv, mv)
                        pp = ppool.tile([128, 2 * QCW], BF16, tag="pp",
                                        name=f"pp_{c}_{p}_{b}")
                        if qs == 0:
                            nc.scalar.activation(pp[:], sp[:], EXP,
                                                 scale=0.125)
                        else:
                            for hh in range(2):
                                nc.scalar.activation(
                                    pp[:, QCW * hh + qs:QCW * (hh + 1)],
                                    sp[:, QCW * hh + qs:QCW * (hh + 1)],
                                    EXP, scale=0.125)
                        pend.append((b, pp, qs))
                        pull_fillers(fpb(b))
                        if len(pend) >= 2:
                            emit_pv()
                    while pend:
                        pull_fillers(1)
                        emit_pv()

                    # normalize (1/s = exp(-ln(s)) on ScalarE) + ship
                    for hh in range(2):
                        h = 2 * p + hh
                        lt = npool.tile([1, QCW], F32, tag="lt",
                                        name=f"lt_{c}_{h}")
                        nc.scalar.activation(lt[0:1, :], ctx2[hh][64:65, :],
                                             LOG)
                        rr = npool.tile([1, QCW], F32, tag="rr",
                                        name=f"rr_{c}_{h}")
                        nc.scalar.activation(rr[0:1, :], lt[0:1, :], EXP,
                                             scale=-1.0)
                        rb = npool.tile([64, QCW], F32, tag="rb",
                                        name=f"rb_{c}_{h}")
                        nc.gpsimd.partition_broadcast(rb[0:64, :], rr[0:1, :])
                        cn = npool.tile([64, QCW], BF16, tag="cn",
                                        name=f"cn_{c}_{h}")
                        nc.vector.tensor_mul(cn[0:64, :], ctx2[hh][0:64, :],
                                             rb[0:64, :])
                        nc.gpsimd.dma_start(
                            out=ag_in[c, p][64 * hh:64 * (hh + 1), :],
                            in_=cn[0:64, :])
                    nc.gpsimd.collective_compute(
                        "AllGather", mybir.AluOpType.bypass,
                        replica_groups=rg,
                        ins=[ag_in[c, p].opt()], outs=[ag_out[c, p].opt()])

                # ---------- out-projection ----------
                outproj_state = {}

                def make_outproj_closures(opsum, c, p):
                    """16 matmul closures (2 gg-groups) for pair (c,p).
                    agt loads are issued NOW (Sync prefetch behind the AG)."""
                    if c not in outproj_state:
                        outproj_state[c] = [
                            opsum.tile([128, QCW], F32, tag="ops",
                                       name=f"ops_{c}_{cb}")
                            for cb in range(2)]
                    ops = outproj_state[c]
                    agts = []
                    for gg in range(2):
                        agt = agpool.tile([128, 4, QCW], BF16, tag="ag",
                                          name=f"agt_{c}_{p}_{gg}")
                        nc.sync.dma_start(
                            out=agt[:],
                            in_=ag_out[c, p].rearrange(
                                "(g q) t -> q g t",
                                q=128)[:, 4 * gg:4 * gg + 4, :])
                        agts.append(agt)
                    cls = []
                    for gg in range(2):
                        for gl in range(4):
                            g = 4 * gg + gl
                            m = 2 * g + p
                            for cb in range(2):
                                def f(gg=gg, gl=gl, m=m, cb=cb, g=g):
                                    nc.tensor.matmul(
                                        ops[cb][:],
                                        wo_sb[:, m, 128 * cb:128 * (cb + 1)],
                                        agts[gg][:, gl, :],
                                        start=(p == 0 and g == 0),
                                        stop=(p == 1 and g == 7))
                                cls.append(f)
                    return cls

                def emit_outproj_finish(c):
                    ops = outproj_state.pop(c)
                    for cb in range(2):
                        osb = npool.tile([128, QCW], F32, tag="osb",
                                         name=f"osb_{c}_{cb}")
                        nc.vector.tensor_copy(osb[:], ops[cb][:])
                        nc.gpsimd.dma_start(
                            out=out[128 * cb:128 * (cb + 1),
                                    QCW * c:QCW * (c + 1)],
                            in_=osb[:])

                # ---------- the schedule ----------
                # scope 1: R0..A3 -- proj 3 banks + vt + spA 2 + cps 2
                with tc.tile_pool(name="projps", bufs=3, space="PSUM") \
                        as projps, \
                     tc.tile_pool(name="tps", bufs=1, space="PSUM") as tps, \
                     tc.tile_pool(name="spsumA", bufs=1, space="PSUM") \
                        as spsumA:

                    # round 0: weights + xg interleaved on Sync/Scalar
                    xgs0 = alloc_xgs(0)
                    r0 = make_proj_closures(projps, 0, xgs0)
                    nc.scalar.dma_start(out=wq_sb[:, 0:4, :],
                                        in_=wq_r[:, 0:4, :])
                    nc.sync.dma_start(out=wkv_sb[:, 0:4, :],
                                      in_=wkv_r[:, 0:4, :])
                    emit_xg_dma(0, xgs0, 0)
                    emit_xg_dma(0, xgs0, 1)     # scalar
                    nc.sync.dma_start(out=wkv_sb[:, 4:8, :],
                                      in_=wkv_r[:, 4:8, :])
                    nc.scalar.dma_start(out=wq_sb[:, 4:8, :],
                                        in_=wq_r[:, 4:8, :])
                    emit_xg_dma(0, xgs0, 2)
                    nc.sync.dma_start(out=wkv_sb[:, 8:12, :],
                                      in_=wkv_r[:, 8:12, :])
                    emit_xg_dma(0, xgs0, 3)     # scalar
                    nc.scalar.dma_start(out=cos_sb[0:64, :], in_=cosT[:])
                    nc.sync.dma_start(out=wkv_sb[:, 12:16, :],
                                      in_=wkv_r[:, 12:16, :])
                    nc.scalar.dma_start(out=sin_sb[0:64, :], in_=sinT[:])
                    nc.sync.dma_start(out=wq_sb[:, 8:12, :],
                                      in_=wq_r[:, 8:12, :])
                    nc.scalar.dma_start(out=wq_sb[:, 12:16, :],
                                        in_=wq_r[:, 12:16, :])
                    nc.gpsimd.dma_start(out=cos_sb[64:128, :],
                                        in_=cos_sb[0:64, :])
                    nc.gpsimd.dma_start(out=sin_sb[64:128, :],
                                        in_=sin_sb[0:64, :])
                    for f in r0:
                        f()
                    emit_copies_rope(0)

                    xgs1 = alloc_xgs(1)
                    for g in range(4):
                        emit_xg_dma(1, xgs1, g)
                    r1 = make_proj_closures(projps, 1, xgs1)
                    for f in r1[:8]:
                        f()
                    fillers.extend(make_vt_closures(tps, range(0, 4)))
                    fillers.extend(r1[8:])
                    pull_fillers(4)

                    emit_attention_pair(spsumA, 0, 0, fpb=4)
                    pull_fillers(99)          # drain R1
                    emit_copies_rope(1)

                    xgs2 = alloc_xgs(2)
                    for g in range(4):
                        emit_xg_dma(2, xgs2, g)
                    fillers.extend(make_proj_closures(projps, 2, xgs2))
                    # wo chunks now (needed from A6 on)
                    for g in range(4):
                        sl = slice(4 * g, 4 * g + 4)
                        nc.sync.dma_start(out=wo_sb[:, sl, :],
                                          in_=wo_r[:, sl, :])
                    emit_attention_pair(spsumA, 0, 1, fpb=4)
                    # keep r2 remainder (32) as A2 fillers; vt4-7 first
                    fillers[:0] = make_vt_closures(tps, range(4, 8))
                    emit_attention_pair(spsumA, 1, 0, fpb=4)
                    pull_fillers(99)          # drain R2 leftovers
                    emit_copies_rope(2)
                    fillers.extend(make_vt_closures(tps, range(8, 12)))

                    xgs3 = alloc_xgs(3)
                    for g in range(4):
                        emit_xg_dma(3, xgs3, g)
                    fillers.extend(make_proj_closures(projps, 3, xgs3))
                    emit_attention_pair(spsumA, 1, 1, fpb=4)
                    pull_fillers(99)          # drain R3
                    emit_copies_rope(3)
                    for f in make_vt_closures(tps, range(12, 16)):
                        f()

                # scope 2: A4..tail -- spB 4 banks + ops 2 + cps 2
                with tc.tile_pool(name="spsumB", bufs=2, space="PSUM") \
                        as spsumB, \
                     tc.tile_pool(name="opsum", bufs=2, space="PSUM") \
                        as opsum:
                    emit_attention_pair(spsumB, 2, 0, fpb=0)
                    emit_attention_pair(spsumB, 2, 1, fpb=0)
                    fillers.extend(make_outproj_closures(opsum, 0, 0))
                    emit_attention_pair(spsumB, 3, 0,
                                        fpb=lambda b: 0 if b < 8 else 2)
                    pull_fillers(99)
                    fillers.extend(make_outproj_closures(opsum, 0, 1))
                    emit_attention_pair(spsumB, 3, 1,
                                        fpb=lambda b: 0 if b < 4 else 2)
                    pull_fillers(99)
                    emit_outproj_finish(0)
                    for f in make_outproj_closures(opsum, 1, 0):
                        f()
                    for f in make_outproj_closures(opsum, 1, 1):
                        f()
                    emit_outproj_finish(1)
                    for f in make_outproj_closures(opsum, 2, 0):
                        f()
                    for f in make_outproj_closures(opsum, 2, 1):
                        f()
                    emit_outproj_finish(2)
                    for f in make_outproj_closures(opsum, 3, 0):
                        f()
                    for f in make_outproj_closures(opsum, 3, 1):
                        f()
                    emit_outproj_finish(3)

    nc.compile()
    return nc


_NC_CACHE = None


def _get_nc():
    global _NC_CACHE
    if _NC_CACHE is None:
        _NC_CACHE = build_nc()
    return _NC_CACHE


def _build_in_maps(inputs):
    import ml_dtypes
    x = np.asarray(inputs["x"], dtype=np.float32)
    Wq = np.asarray(inputs["Wq"], dtype=np.float32)
    Wk = np.asarray(inputs["Wk"], dtype=np.float32)
    Wv = np.asarray(inputs["Wv"], dtype=np.float32)
    Wo = np.asarray(inputs["Wo"], dtype=np.float32)
    sp = int(np.asarray(inputs["start_pos"]))

    b, t, d = x.shape
    assert (b, t, d) == (1, T, D), (b, t, d)

    cosT_np, sinT_np = _rope_tables(sp)
    xT_r = np.ascontiguousarray(x[0].T).astype(ml_dtypes.bfloat16)  # [D, T]
    tri = np.where(np.arange(KBW)[:, None] > np.arange(KBW)[None, :],
                   np.float32(-1e30), np.float32(0.0))
    mask2_np = np.ascontiguousarray(np.tile(tri, (1, 2)))  # [128, 256]
    eye_np = np.tile(np.eye(64, dtype=np.float32), (2, 1))  # [128, 64]

    in_maps = []
    for i in range(N_CORES):
        wqT_i = np.ascontiguousarray(
            Wq[DLOC * i:DLOC * (i + 1), :].T).astype(ml_dtypes.bfloat16)
        wkv_i = np.ascontiguousarray(
            np.concatenate([Wk[HD * i:HD * (i + 1), :].T,
                            Wv[HD * i:HD * (i + 1), :].T],
                           axis=1)).astype(ml_dtypes.bfloat16)  # [D, 128]
        woT_i = np.ascontiguousarray(
            Wo[DLOC * i:DLOC * (i + 1), :].T).astype(ml_dtypes.bfloat16)
        in_maps.append({
            "xT": xT_r,
            "wqT": wqT_i,
            "wkvT": wkv_i,
            "woT": woT_i,
            "cosT": cosT_np,
            "sinT": sinT_np,
            "mask2": mask2_np,
            "eye64": eye_np,
        })
    return in_maps


def kernel(x, Wq, Wk, Wv, Wo, start_pos):
    in_maps = _build_in_maps(dict(x=x, Wq=Wq, Wk=Wk, Wv=Wv, Wo=Wo,
                                  start_pos=start_pos))
    nc = _get_nc()
    res = run_bass_kernel_spmd(nc, in_maps, core_ids=list(range(N_CORES)))

    outT = np.empty((T, D), dtype=np.float32)
    for i in range(N_CORES):
        outT[:, DLOC * i:DLOC * (i + 1)] = res.results[i]["out"].T
    return outT[None, :, :]


if __name__ == "__main__":
    rng = np.random.default_rng(0)
    inputs = {
        "x": rng.standard_normal((1, T, D)).astype(np.float32),
        "Wq": (rng.standard_normal((D, D)) * 0.02).astype(np.float32),
        "Wk": (rng.standard_normal((512, D)) * 0.02).astype(np.float32),
        "Wv": (rng.standard_normal((512, D)) * 0.02).astype(np.float32),
        "Wo": (rng.standard_normal((D, D)) * 0.02).astype(np.float32),
        "start_pos": 0,
    }
    y = kernel(**inputs)
    print("kernel output shape:", y.shape, "finite:", np.isfinite(y).all())
